# revision 9
# baseline (speedup 1.0000x reference)
"""MoE FFN (8 experts, top-2) on 8 Trainium2 NeuronCores.

Expert parallelism with half-expert load balancing: the router runs on host
(same jax ops as the reference); each expert's FFN is split along the hidden
dim into two halves computed on two different cores, and each core serves one
half of a *large* expert (segment A) plus one half of a *small* expert
(segment B), pairing rank k with rank 7-k by token count. This caps per-core
work at (max_large + max_small)/2 token-equivalents instead of max over all
experts. The host sums the two half-expert partial outputs, adds b2, applies
the combine weights, and scatter-adds into the final output.

On-device layout: contraction dim lives on SBUF partitions for every matmul.
Weights are host-prearranged so each consumed [128,128] stationary block
arrives as part of a single [128 x 2KB-line] descriptor in exact consumption
order (m-major), and the initial loads are spread across three engine DMA
queues (sync/scalar/gpsimd) so the first matmul isn't gated on one queue's
cold-start ramp. PSUM accumulates f32; the layer-1 bias rides the gelu on
ScalarE; layer-2 output is evicted to bf16 (b2 is added on host). Layer 2's
contraction is issued in two halves so the last-gelu latency hides under the
first half's matmuls instead of stalling the PE at each tile boundary.
"""

import numpy as np
import ml_dtypes

N_EXPERTS = 8
TOP_K = 2
C = 1024
H = 4096
HH = H // 2
P = 128
T_TILE = 512
KO1 = C // P   # 8 contraction chunks for layer 1
MH = HH // P   # 16 hidden chunks per half-expert

_nc_cache = {}


def _split_tiles(cap: int, ramp: bool = False):
    # Balanced token tiles in P units; every tile <= T_TILE and big enough to
    # stay above the LDWEIGHTS floor. With ramp=True the first tiles are small
    # (256/384) so the PE starts while the cold DMA queues are still slow.
    if ramp and cap >= 1152:
        head = [256, 384]
        rest = _split_tiles(cap - 640)
        return head + rest
    n_tiles = -(-cap // T_TILE)
    k = cap // P
    tiles = [(k // n_tiles + (1 if i < k % n_tiles else 0)) * P for i in range(n_tiles)]
    assert sum(tiles) == cap and all(t <= T_TILE for t in tiles)
    return tiles


def _build_nc(capA: int, capB: int):
    import concourse.mybir as mybir
    import concourse.tile as tile
    from concourse import bacc

    bf16 = mybir.dt.bfloat16
    f32 = mybir.dt.float32
    gelu = mybir.ActivationFunctionType.Gelu_apprx_tanh

    nc = bacc.Bacc()
    dram = {}
    for s, cap in (("A", capA), ("B", capB)):
        dram[f"xt{s}"] = nc.dram_tensor(f"xt{s}", [C, cap], bf16, kind="ExternalInput")
        # weights host-prearranged: [ki, m, ko*128+j] = w1[m*128+j, ko*128+ki]
        dram[f"w1t{s}"] = nc.dram_tensor(f"w1t{s}", [P, MH, C], bf16, kind="ExternalInput")
        dram[f"w2t{s}"] = nc.dram_tensor(f"w2t{s}", [P, MH, C], bf16, kind="ExternalInput")
        dram[f"b1{s}"] = nc.dram_tensor(f"b1{s}", [P, MH], f32, kind="ExternalInput")
        dram[f"yt{s}"] = nc.dram_tensor(f"yt{s}", [C, cap], bf16, kind="ExternalOutput")

    xr = {s: dram[f"xt{s}"].rearrange("(ko ki) t -> ki ko t", ki=P) for s in "AB"}
    yr = {s: dram[f"yt{s}"].rearrange("(co p) t -> p co t", p=P) for s in "AB"}

    # smallest tile first in A: less x to load before the first matmul while
    # the DMA engines are still ramping.
    tiles = {"A": _split_tiles(capA, ramp=True), "B": _split_tiles(capB)}
    sched = []
    for s in "AB":
        t0 = 0
        for ti, T in enumerate(tiles[s]):
            sched.append((s, ti, T, t0))
            t0 += T

    with tile.TileContext(nc) as tc:
        with (
            tc.tile_pool(name="const", bufs=1) as const,
            tc.tile_pool(name="xp", bufs=2) as xp,
            tc.tile_pool(name="gp", bufs=1) as gp,
            tc.tile_pool(name="yp", bufs=4) as yp,
            tc.tile_pool(name="psum", bufs=8, space="PSUM") as psum,
        ):
            w1_sb = {
                s: const.tile([P, MH, C], bf16, tag=f"w1{s}", name=f"w1{s}")
                for s in "AB"
            }
            w2_sb = {
                s: const.tile([P, MH, C], bf16, tag=f"w2{s}", name=f"w2{s}")
                for s in "AB"
            }
            b1_sb = {
                s: const.tile([P, MH], f32, tag=f"b1{s}", name=f"b1{s}")
                for s in "AB"
            }

            # --- initial loads, spread across the two HW DGE queues ---
            # sync: x tiles; scalar: b1A + segment-A weight stream in
            # consumption order. Both queues only come alive ~9us in, so the
            # first tile is small (256 tokens) to start the PE early.
            x_tiles = {}
            T0 = tiles["A"][0]
            x_tiles[0] = xp.tile([P, KO1, T_TILE], bf16, tag="x", name="x0")
            for ko in range(KO1):
                nc.sync.dma_start(x_tiles[0][:, ko : ko + 1, :T0], xr["A"][:, ko : ko + 1, :T0])
            nc.scalar.dma_start(b1_sb["A"][:], dram["b1A"][:])
            for m in range(MH):
                nc.scalar.dma_start(
                    w1_sb["A"][:, m : m + 1, :], dram["w1tA"][:, m : m + 1, :]
                )
            for ho in range(MH):
                nc.scalar.dma_start(
                    w2_sb["A"][:, ho : ho + 1, :], dram["w2tA"][:, ho : ho + 1, :]
                )

            for gi, (s, ti, T, t0) in enumerate(sched):
                # prefetch next tile's x (paced by the xp pool)
                if gi + 1 < len(sched):
                    ns, nti, nT, nt0 = sched[gi + 1]
                    x_tiles[gi + 1] = xp.tile(
                        [P, KO1, T_TILE], bf16, tag="x", name=f"x{gi + 1}"
                    )
                    nc.sync.dma_start(
                        x_tiles[gi + 1][:, :, :nT], xr[ns][:, :, nt0 : nt0 + nT]
                    )
                if gi == 2:
                    # segment-B constants stream on the (slow, software) gpsimd
                    # queue while segment A computes -- it has ~200us of slack.
                    nc.gpsimd.dma_start(b1_sb["B"][:], dram["b1B"][:])
                    for m in range(MH):
                        nc.gpsimd.dma_start(
                            w1_sb["B"][:, m : m + 1, :], dram["w1tB"][:, m : m + 1, :]
                        )
                    for ho in range(MH):
                        nc.gpsimd.dma_start(
                            w2_sb["B"][:, ho : ho + 1, :], dram["w2tB"][:, ho : ho + 1, :]
                        )

                x_sb = x_tiles.pop(gi)
                g_sb = gp.tile([P, MH, T_TILE], bf16, tag="g", name=f"g{gi}")
                for m in range(MH):
                    ph = psum.tile([P, T_TILE], f32, tag="ps", name=f"ph{gi}_{m}")
                    for ko in range(KO1):
                        nc.tensor.matmul(
                            ph[:, :T],
                            w1_sb[s][:, m, ko * P : (ko + 1) * P],
                            x_sb[:, ko, :T],
                            start=(ko == 0),
                            stop=(ko == KO1 - 1),
                        )
                    nc.scalar.activation(
                        g_sb[:, m, :T], ph[:, :T], gelu, bias=b1_sb[s][:, m : m + 1]
                    )
                # layer 2, contraction split in two: the first half only needs
                # g[:8], so it issues right behind the L1 matmuls while the
                # m=15 gelu drains; the second half lands long after.
                pys = [
                    psum.tile([P, T_TILE], f32, tag="ps", name=f"py{gi}_{co}")
                    for co in range(KO1)
                ]
                for co in range(KO1):
                    for ho in range(MH // 2):
                        nc.tensor.matmul(
                            pys[co][:, :T],
                            w2_sb[s][:, ho, co * P : (co + 1) * P],
                            g_sb[:, ho, :T],
                            start=(ho == 0),
                            stop=False,
                        )
                for co in range(KO1):
                    for ho in range(MH // 2, MH):
                        nc.tensor.matmul(
                            pys[co][:, :T],
                            w2_sb[s][:, ho, co * P : (co + 1) * P],
                            g_sb[:, ho, :T],
                            start=False,
                            stop=(ho == MH - 1),
                        )
                    y_sb = yp.tile([P, T_TILE], bf16, tag="y", name=f"y{gi}_{co}")
                    nc.vector.tensor_copy(y_sb[:, :T], pys[co][:, :T])
                    nc.sync.dma_start(yr[s][:, co, t0 : t0 + T], y_sb[:, :T])
    nc.finalize()
    return nc


def _route(flat_f32: np.ndarray, gate_w: np.ndarray):
    """Router, bit-matching the reference's jax ops (same env/backend)."""
    import jax
    import jax.numpy as jnp

    logits = jnp.asarray(flat_f32) @ jnp.asarray(gate_w).T
    probs = jax.nn.softmax(logits, axis=-1)
    top_p, top_i = jax.lax.top_k(probs, TOP_K)
    weights = top_p / (jnp.sum(top_p, axis=-1, keepdims=True) + 1e-8)
    return np.asarray(top_i), np.asarray(weights)


# results of the last device run, for test harness introspection
last_result = None


def _ensure_ntff_hook():
    """bass_utils' trace path imports antenv.axon_hooks, which the agent
    image's antenv lacks. Build the hook from trn_agent_boot's ctypes
    shim and inject a stand-in module."""
    import sys
    import types

    if "antenv.axon_hooks" in sys.modules:
        return
    try:
        from trn_agent_boot.trn_boot import _ntff_profile_via_ctypes

        hook = _ntff_profile_via_ctypes("/opt/axon/libaxon_pjrt.so")
    except Exception:
        hook = None
    m = types.ModuleType("antenv.axon_hooks")
    m.get_axon_ntff_profile_hook = lambda: hook
    m.set_axon_ntff_profile_hook = lambda h: None
    sys.modules["antenv.axon_hooks"] = m


def _prep_w1(w1e_half: np.ndarray) -> np.ndarray:
    # [HH, C] -> [ki, m, ko*128+j] with value w1[m*128+j, ko*128+ki]
    bf16 = ml_dtypes.bfloat16
    a = w1e_half.reshape(MH, P, KO1, P)          # [m, j, ko, ki]
    a = a.transpose(3, 0, 2, 1).reshape(P, MH, C)  # [ki, m, (ko j)]
    return np.ascontiguousarray(a).astype(bf16)


def _prep_w2(w2e_half: np.ndarray) -> np.ndarray:
    # [C, HH] -> [ki, ho, co*128+j] with value w2[co*128+j, ho*128+ki]
    bf16 = ml_dtypes.bfloat16
    a = w2e_half.reshape(KO1, P, MH, P)          # [co, j, ho, ki]
    a = a.transpose(3, 2, 0, 1).reshape(P, MH, C)  # [ki, ho, (co j)]
    return np.ascontiguousarray(a).astype(bf16)


def kernel(x, gate_w, w1, b1, w2, b2):
    from concourse.bass_utils import run_bass_kernel_spmd

    x = np.asarray(x)
    B, N, _ = x.shape
    flat = np.ascontiguousarray(x.reshape(-1, C), dtype=np.float32)
    w1 = np.asarray(w1, dtype=np.float32)
    w2 = np.asarray(w2, dtype=np.float32)
    b1 = np.asarray(b1, dtype=np.float32)
    b2 = np.asarray(b2, dtype=np.float32)

    top_i, weights = _route(flat, np.asarray(gate_w, dtype=np.float32))

    # token ids and combine weights per expert
    idx_e, g_e = [], []
    for e in range(N_EXPERTS):
        rows, cols = np.nonzero(top_i == e)
        idx_e.append(rows)
        g_e.append(weights[rows, cols].astype(np.float32))
    counts = np.array([len(i) for i in idx_e])

    # rank experts by load; segment A = big four, B = small four. Expert
    # ranked[r] runs as two hidden-halves on cores r and r+4; expert
    # ranked[7-r] likewise (segment B on the same core pair).
    ranked = np.argsort(-counts, kind="stable")
    pad = lambda n: max(int(-(-n // P) * P), P)
    capA = pad(int(counts[ranked[0]]))
    capB = pad(int(counts[ranked[4]]))

    key = (capA, capB)
    nc = _nc_cache.get(key)
    if nc is None:
        nc = _build_nc(capA, capB)
        _nc_cache[key] = nc

    bf16 = ml_dtypes.bfloat16

    # per-expert padded x (shared by the expert's two half-cores)
    xt = {}
    for s, cap, exps in (("A", capA, ranked[:4]), ("B", capB, ranked[4:])):
        for e in exps:
            xe = np.zeros((C, cap), dtype=bf16)
            xe[:, : counts[e]] = flat[idx_e[e]].T.astype(bf16)
            xt[int(e)] = xe

    in_maps = []
    for core in range(8):
        r, half = core % 4, core // 4
        m = {}
        for s, r_e in (("A", ranked[r]), ("B", ranked[7 - r])):
            e = int(r_e)
            lo, hi = half * HH, (half + 1) * HH
            m[f"xt{s}"] = xt[e]
            m[f"w1t{s}"] = _prep_w1(w1[e, lo:hi, :])
            m[f"w2t{s}"] = _prep_w2(w2[e, :, lo:hi])
            m[f"b1{s}"] = np.ascontiguousarray(b1[e, lo:hi].reshape(MH, P).T)
        in_maps.append(m)

    import os

    trace = bool(int(os.environ.get("MOE_TRACE", "0")))
    if trace:
        _ensure_ntff_hook()

    global last_result
    res = run_bass_kernel_spmd(
        nc,
        in_maps,
        core_ids=list(range(8)),
        trace=trace,
    )
    last_result = res

    out = np.zeros((flat.shape[0], C), dtype=np.float32)
    for r in range(4):
        for s, r_e in (("A", ranked[r]), ("B", ranked[7 - r])):
            e = int(r_e)
            cnt = counts[e]
            y = res.results[r][f"yt{s}"][:, :cnt].astype(np.float32)
            y += res.results[r + 4][f"yt{s}"][:, :cnt].astype(np.float32)
            out[idx_e[e]] += g_e[e][:, None] * (y.T + b2[e])
    return out.reshape(B, N, C)


# revision 11
# speedup vs baseline: 1.0762x; 1.0762x over previous
"""MoE FFN (8 experts, top-2) on 8 Trainium2 NeuronCores.

Expert parallelism with half-expert load balancing: the router runs on host
(same jax ops as the reference); each expert's FFN is split along the hidden
dim into two halves computed on two different cores, and each core serves one
half of a *large* expert (segment A) plus one half of a *small* expert
(segment B), pairing rank k with rank 7-k by token count. This caps per-core
work at (max_large + max_small)/2 token-equivalents instead of max over all
experts. The host sums the two half-expert partial outputs, adds b2, applies
the combine weights, and scatter-adds into the final output.

On-device layout: contraction dim lives on SBUF partitions for every matmul.
Weights are host-prearranged so each consumed [128,128] stationary block
arrives as part of a single [128 x 2KB-line] descriptor in exact consumption
order (m-major), and the initial loads are spread across three engine DMA
queues (sync/scalar/gpsimd) so the first matmul isn't gated on one queue's
cold-start ramp. PSUM accumulates f32; the layer-1 bias rides the gelu on
ScalarE; layer-2 output is evicted to bf16 (b2 is added on host). Layer 2's
contraction is issued in two halves so the last-gelu latency hides under the
first half's matmuls instead of stalling the PE at each tile boundary.
"""

import numpy as np
import ml_dtypes

N_EXPERTS = 8
TOP_K = 2
C = 1024
H = 4096
HH = H // 2
P = 128
T_TILE = 512
KO1 = C // P   # 8 contraction chunks for layer 1
MH = HH // P   # 16 hidden chunks per half-expert

_nc_cache = {}


def _split_tiles(cap: int, ramp: bool = False):
    # Balanced token tiles in P units; every tile <= T_TILE and big enough to
    # stay above the LDWEIGHTS floor. With ramp=True the first tiles are small
    # (256/384) so the PE starts while the cold DMA queues are still slow.
    if ramp and cap >= 1152:
        head = [256, 384]
        rest = _split_tiles(cap - 640)
        return head + rest
    n_tiles = -(-cap // T_TILE)
    k = cap // P
    tiles = [(k // n_tiles + (1 if i < k % n_tiles else 0)) * P for i in range(n_tiles)]
    assert sum(tiles) == cap and all(t <= T_TILE for t in tiles)
    return tiles


def _build_nc(capA: int, capB: int):
    import concourse.mybir as mybir
    import concourse.tile as tile
    from concourse import bacc

    bf16 = mybir.dt.bfloat16
    f32 = mybir.dt.float32
    gelu = mybir.ActivationFunctionType.Gelu_apprx_tanh

    nc = bacc.Bacc()
    dram = {}
    for s, cap in (("A", capA), ("B", capB)):
        dram[f"xt{s}"] = nc.dram_tensor(f"xt{s}", [C, cap], bf16, kind="ExternalInput")
        # weights host-prearranged: [ki, m, ko*128+j] = w1[m*128+j, ko*128+ki]
        dram[f"w1t{s}"] = nc.dram_tensor(f"w1t{s}", [P, MH, C], bf16, kind="ExternalInput")
        dram[f"w2t{s}"] = nc.dram_tensor(f"w2t{s}", [P, MH, C], bf16, kind="ExternalInput")
        dram[f"b1{s}"] = nc.dram_tensor(f"b1{s}", [P, MH], f32, kind="ExternalInput")
        dram[f"yt{s}"] = nc.dram_tensor(f"yt{s}", [C, cap], bf16, kind="ExternalOutput")

    xr = {s: dram[f"xt{s}"].rearrange("(ko ki) t -> ki ko t", ki=P) for s in "AB"}
    yr = {s: dram[f"yt{s}"].rearrange("(co p) t -> p co t", p=P) for s in "AB"}

    # smallest tile first in A: less x to load before the first matmul while
    # the DMA engines are still ramping.
    tiles = {"A": sorted(_split_tiles(capA)), "B": _split_tiles(capB)}
    sched = []
    for s in "AB":
        t0 = 0
        for ti, T in enumerate(tiles[s]):
            sched.append((s, ti, T, t0))
            t0 += T

    with tile.TileContext(nc) as tc:
        with (
            tc.tile_pool(name="const", bufs=1) as const,
            tc.tile_pool(name="xp", bufs=2) as xp,
            tc.tile_pool(name="gp", bufs=1) as gp,
            tc.tile_pool(name="yp", bufs=4) as yp,
            tc.tile_pool(name="psum", bufs=8, space="PSUM") as psum,
        ):
            w1_sb = {
                s: const.tile([P, MH, C], bf16, tag=f"w1{s}", name=f"w1{s}")
                for s in "AB"
            }
            w2_sb = {
                s: const.tile([P, MH, C], bf16, tag=f"w2{s}", name=f"w2{s}")
                for s in "AB"
            }
            b1_sb = {
                s: const.tile([P, MH], f32, tag=f"b1{s}", name=f"b1{s}")
                for s in "AB"
            }

            # --- initial loads, spread across queues ---
            # sync/scalar (HW DGE): x0 halves + b1A; gpsimd: the segment-A
            # weight stream in consumption order. Putting a multi-MB stream on
            # the scalar queue would block the scalar engine on ring slots and
            # stall every activation behind it, so weights go to gpsimd.
            x_tiles = {}
            T0 = tiles["A"][0]
            x_tiles[0] = xp.tile([P, KO1, T_TILE], bf16, tag="x", name="x0")
            for ko in range(KO1):
                eng = nc.sync if ko < 4 else nc.scalar
                eng.dma_start(x_tiles[0][:, ko : ko + 1, :T0], xr["A"][:, ko : ko + 1, :T0])
            nc.scalar.dma_start(b1_sb["A"][:], dram["b1A"][:])
            for m in range(MH):
                nc.gpsimd.dma_start(
                    w1_sb["A"][:, m : m + 1, :], dram["w1tA"][:, m : m + 1, :]
                )
            for ho in range(MH):
                nc.gpsimd.dma_start(
                    w2_sb["A"][:, ho : ho + 1, :], dram["w2tA"][:, ho : ho + 1, :]
                )

            for gi, (s, ti, T, t0) in enumerate(sched):
                # prefetch next tile's x (paced by the xp pool)
                if gi + 1 < len(sched):
                    ns, nti, nT, nt0 = sched[gi + 1]
                    x_tiles[gi + 1] = xp.tile(
                        [P, KO1, T_TILE], bf16, tag="x", name=f"x{gi + 1}"
                    )
                    nc.sync.dma_start(
                        x_tiles[gi + 1][:, :, :nT], xr[ns][:, :, nt0 : nt0 + nT]
                    )
                if gi == 2:
                    # segment-B constants stream on the (slow, software) gpsimd
                    # queue while segment A computes -- it has ~200us of slack.
                    nc.gpsimd.dma_start(b1_sb["B"][:], dram["b1B"][:])
                    for m in range(MH):
                        nc.gpsimd.dma_start(
                            w1_sb["B"][:, m : m + 1, :], dram["w1tB"][:, m : m + 1, :]
                        )
                    for ho in range(MH):
                        nc.gpsimd.dma_start(
                            w2_sb["B"][:, ho : ho + 1, :], dram["w2tB"][:, ho : ho + 1, :]
                        )

                x_sb = x_tiles.pop(gi)
                g_sb = gp.tile([P, MH, T_TILE], bf16, tag="g", name=f"g{gi}")
                for m in range(MH):
                    ph = psum.tile([P, T_TILE], f32, tag="ps", name=f"ph{gi}_{m}")
                    for ko in range(KO1):
                        nc.tensor.matmul(
                            ph[:, :T],
                            w1_sb[s][:, m, ko * P : (ko + 1) * P],
                            x_sb[:, ko, :T],
                            start=(ko == 0),
                            stop=(ko == KO1 - 1),
                        )
                    nc.scalar.activation(
                        g_sb[:, m, :T], ph[:, :T], gelu, bias=b1_sb[s][:, m : m + 1]
                    )
                # layer 2, contraction split in two: the first half only needs
                # g[:8], so it issues right behind the L1 matmuls while the
                # m=15 gelu drains; the second half lands long after.
                pys = [
                    psum.tile([P, T_TILE], f32, tag="ps", name=f"py{gi}_{co}")
                    for co in range(KO1)
                ]
                for co in range(KO1):
                    for ho in range(MH // 2):
                        nc.tensor.matmul(
                            pys[co][:, :T],
                            w2_sb[s][:, ho, co * P : (co + 1) * P],
                            g_sb[:, ho, :T],
                            start=(ho == 0),
                            stop=False,
                        )
                for co in range(KO1):
                    for ho in range(MH // 2, MH):
                        nc.tensor.matmul(
                            pys[co][:, :T],
                            w2_sb[s][:, ho, co * P : (co + 1) * P],
                            g_sb[:, ho, :T],
                            start=False,
                            stop=(ho == MH - 1),
                        )
                    y_sb = yp.tile([P, T_TILE], bf16, tag="y", name=f"y{gi}_{co}")
                    nc.vector.tensor_copy(y_sb[:, :T], pys[co][:, :T])
                    nc.sync.dma_start(yr[s][:, co, t0 : t0 + T], y_sb[:, :T])
    nc.finalize()
    return nc


def _route(flat_f32: np.ndarray, gate_w: np.ndarray):
    """Router, bit-matching the reference's jax ops (same env/backend)."""
    import jax
    import jax.numpy as jnp

    logits = jnp.asarray(flat_f32) @ jnp.asarray(gate_w).T
    probs = jax.nn.softmax(logits, axis=-1)
    top_p, top_i = jax.lax.top_k(probs, TOP_K)
    weights = top_p / (jnp.sum(top_p, axis=-1, keepdims=True) + 1e-8)
    return np.asarray(top_i), np.asarray(weights)


# results of the last device run, for test harness introspection
last_result = None


def _ensure_ntff_hook():
    """bass_utils' trace path imports antenv.axon_hooks, which the agent
    image's antenv lacks. Build the hook from trn_agent_boot's ctypes
    shim and inject a stand-in module."""
    import sys
    import types

    if "antenv.axon_hooks" in sys.modules:
        return
    try:
        from trn_agent_boot.trn_boot import _ntff_profile_via_ctypes

        hook = _ntff_profile_via_ctypes("/opt/axon/libaxon_pjrt.so")
    except Exception:
        hook = None
    m = types.ModuleType("antenv.axon_hooks")
    m.get_axon_ntff_profile_hook = lambda: hook
    m.set_axon_ntff_profile_hook = lambda h: None
    sys.modules["antenv.axon_hooks"] = m


def _prep_w1(w1e_half: np.ndarray) -> np.ndarray:
    # [HH, C] -> [ki, m, ko*128+j] with value w1[m*128+j, ko*128+ki]
    bf16 = ml_dtypes.bfloat16
    a = w1e_half.reshape(MH, P, KO1, P)          # [m, j, ko, ki]
    a = a.transpose(3, 0, 2, 1).reshape(P, MH, C)  # [ki, m, (ko j)]
    return np.ascontiguousarray(a).astype(bf16)


def _prep_w2(w2e_half: np.ndarray) -> np.ndarray:
    # [C, HH] -> [ki, ho, co*128+j] with value w2[co*128+j, ho*128+ki]
    bf16 = ml_dtypes.bfloat16
    a = w2e_half.reshape(KO1, P, MH, P)          # [co, j, ho, ki]
    a = a.transpose(3, 2, 0, 1).reshape(P, MH, C)  # [ki, ho, (co j)]
    return np.ascontiguousarray(a).astype(bf16)


def kernel(x, gate_w, w1, b1, w2, b2):
    from concourse.bass_utils import run_bass_kernel_spmd

    x = np.asarray(x)
    B, N, _ = x.shape
    flat = np.ascontiguousarray(x.reshape(-1, C), dtype=np.float32)
    w1 = np.asarray(w1, dtype=np.float32)
    w2 = np.asarray(w2, dtype=np.float32)
    b1 = np.asarray(b1, dtype=np.float32)
    b2 = np.asarray(b2, dtype=np.float32)

    top_i, weights = _route(flat, np.asarray(gate_w, dtype=np.float32))

    # token ids and combine weights per expert
    idx_e, g_e = [], []
    for e in range(N_EXPERTS):
        rows, cols = np.nonzero(top_i == e)
        idx_e.append(rows)
        g_e.append(weights[rows, cols].astype(np.float32))
    counts = np.array([len(i) for i in idx_e])

    # rank experts by load; segment A = big four, B = small four. Expert
    # ranked[r] runs as two hidden-halves on cores r and r+4; expert
    # ranked[7-r] likewise (segment B on the same core pair).
    ranked = np.argsort(-counts, kind="stable")
    pad = lambda n: max(int(-(-n // P) * P), P)
    capA = pad(int(counts[ranked[0]]))
    capB = pad(int(counts[ranked[4]]))

    key = (capA, capB)
    nc = _nc_cache.get(key)
    if nc is None:
        nc = _build_nc(capA, capB)
        _nc_cache[key] = nc

    bf16 = ml_dtypes.bfloat16

    # per-expert padded x (shared by the expert's two half-cores)
    xt = {}
    for s, cap, exps in (("A", capA, ranked[:4]), ("B", capB, ranked[4:])):
        for e in exps:
            xe = np.zeros((C, cap), dtype=bf16)
            xe[:, : counts[e]] = flat[idx_e[e]].T.astype(bf16)
            xt[int(e)] = xe

    in_maps = []
    for core in range(8):
        r, half = core % 4, core // 4
        m = {}
        for s, r_e in (("A", ranked[r]), ("B", ranked[7 - r])):
            e = int(r_e)
            lo, hi = half * HH, (half + 1) * HH
            m[f"xt{s}"] = xt[e]
            m[f"w1t{s}"] = _prep_w1(w1[e, lo:hi, :])
            m[f"w2t{s}"] = _prep_w2(w2[e, :, lo:hi])
            m[f"b1{s}"] = np.ascontiguousarray(b1[e, lo:hi].reshape(MH, P).T)
        in_maps.append(m)

    import os

    trace = bool(int(os.environ.get("MOE_TRACE", "0")))
    if trace:
        _ensure_ntff_hook()

    global last_result
    res = run_bass_kernel_spmd(
        nc,
        in_maps,
        core_ids=list(range(8)),
        trace=trace,
    )
    last_result = res

    out = np.zeros((flat.shape[0], C), dtype=np.float32)
    for r in range(4):
        for s, r_e in (("A", ranked[r]), ("B", ranked[7 - r])):
            e = int(r_e)
            cnt = counts[e]
            y = res.results[r][f"yt{s}"][:, :cnt].astype(np.float32)
            y += res.results[r + 4][f"yt{s}"][:, :cnt].astype(np.float32)
            out[idx_e[e]] += g_e[e][:, None] * (y.T + b2[e])
    return out.reshape(B, N, C)


# revision 14
# speedup vs baseline: 1.0810x; 1.0045x over previous
"""MoE FFN (8 experts, top-2) on 8 Trainium2 NeuronCores.

Expert parallelism with half-expert load balancing: the router runs on host
(same jax ops as the reference); each expert's FFN is split along the hidden
dim into two halves computed on two different cores, and each core serves one
half of a *large* expert (segment A) plus one half of a *small* expert
(segment B), pairing rank k with rank 7-k by token count. This caps per-core
work at (max_large + max_small)/2 token-equivalents instead of max over all
experts. The host sums the two half-expert partial outputs, adds b2, applies
the combine weights, and scatter-adds into the final output.

On-device layout: contraction dim lives on SBUF partitions for every matmul.
Weights are host-prearranged so each consumed [128,128] stationary block
arrives as part of a single [128 x 2KB-line] descriptor in exact consumption
order (m-major), and the initial loads are spread across three engine DMA
queues (sync/scalar/gpsimd) so the first matmul isn't gated on one queue's
cold-start ramp. PSUM accumulates f32; the layer-1 bias rides the gelu on
ScalarE; layer-2 output is evicted to bf16 (b2 is added on host). Layer 2's
contraction is issued in two halves so the last-gelu latency hides under the
first half's matmuls instead of stalling the PE at each tile boundary.
"""

import numpy as np
import ml_dtypes

N_EXPERTS = 8
TOP_K = 2
C = 1024
H = 4096
HH = H // 2
P = 128
T_TILE = 512
KO1 = C // P   # 8 contraction chunks for layer 1
MH = HH // P   # 16 hidden chunks per half-expert

_nc_cache = {}


def _split_tiles(cap: int, ramp: bool = False):
    # Balanced token tiles in P units; every tile <= T_TILE and big enough to
    # stay above the LDWEIGHTS floor. With ramp=True the first tiles are small
    # (256/384) so the PE starts while the cold DMA queues are still slow.
    if ramp and cap >= 1152:
        head = [256, 384]
        rest = _split_tiles(cap - 640)
        return head + rest
    n_tiles = -(-cap // T_TILE)
    k = cap // P
    tiles = [(k // n_tiles + (1 if i < k % n_tiles else 0)) * P for i in range(n_tiles)]
    assert sum(tiles) == cap and all(t <= T_TILE for t in tiles)
    return tiles


def _build_nc(capA: int, capB: int):
    import concourse.mybir as mybir
    import concourse.tile as tile
    from concourse import bacc

    bf16 = mybir.dt.bfloat16
    f32 = mybir.dt.float32
    gelu = mybir.ActivationFunctionType.Gelu_apprx_tanh

    nc = bacc.Bacc()
    dram = {}
    for s, cap in (("A", capA), ("B", capB)):
        dram[f"xt{s}"] = nc.dram_tensor(f"xt{s}", [C, cap], bf16, kind="ExternalInput")
        # weights host-prearranged: [ki, m, ko*128+j] = w1[m*128+j, ko*128+ki]
        dram[f"w1t{s}"] = nc.dram_tensor(f"w1t{s}", [P, MH, C], bf16, kind="ExternalInput")
        dram[f"w2t{s}"] = nc.dram_tensor(f"w2t{s}", [P, MH, C], bf16, kind="ExternalInput")
        dram[f"b1{s}"] = nc.dram_tensor(f"b1{s}", [P, MH], f32, kind="ExternalInput")
        dram[f"yt{s}"] = nc.dram_tensor(f"yt{s}", [C, cap], bf16, kind="ExternalOutput")

    xr = {s: dram[f"xt{s}"].rearrange("(ko ki) t -> ki ko t", ki=P) for s in "AB"}
    yr = {s: dram[f"yt{s}"].rearrange("(co p) t -> p co t", p=P) for s in "AB"}

    # small tiles first in A: less x to load before the first matmul while
    # the DMA engines are still ramping.
    tiles = {"A": _split_tiles(capA, ramp=True), "B": _split_tiles(capB)}
    sched = []
    for s in "AB":
        t0 = 0
        for ti, T in enumerate(tiles[s]):
            sched.append((s, ti, T, t0))
            t0 += T

    with tile.TileContext(nc) as tc:
        with (
            tc.tile_pool(name="const", bufs=1) as const,
            tc.tile_pool(name="xp", bufs=2) as xp,
            tc.tile_pool(name="gp", bufs=1) as gp,
            tc.tile_pool(name="yp", bufs=6) as yp,
            tc.tile_pool(name="psum", bufs=8, space="PSUM") as psum,
        ):
            w1_sb = {
                s: const.tile([P, MH, C], bf16, tag=f"w1{s}", name=f"w1{s}")
                for s in "AB"
            }
            w2_sb = {
                s: const.tile([P, MH, C], bf16, tag=f"w2{s}", name=f"w2{s}")
                for s in "AB"
            }
            b1_sb = {
                s: const.tile([P, MH], f32, tag=f"b1{s}", name=f"b1{s}")
                for s in "AB"
            }

            # --- initial loads, spread across queues ---
            # sync/scalar (HW DGE): x0 halves + b1A; gpsimd: the segment-A
            # weight stream in consumption order. Putting a multi-MB stream on
            # the scalar queue would block the scalar engine on ring slots and
            # stall every activation behind it, so weights go to gpsimd.
            x_tiles = {}
            T0 = tiles["A"][0]
            x_tiles[0] = xp.tile([P, KO1, T_TILE], bf16, tag="x", name="x0")
            nc.sync.dma_start(x_tiles[0][:, :, :T0], xr["A"][:, :, :T0])
            nc.scalar.dma_start(b1_sb["A"][:], dram["b1A"][:])
            # first two m-chunks ride the fast scalar HW queue; the rest of
            # the segment-A stream goes to gpsimd so the scalar ring stays
            # clear for activations.
            for m in range(2):
                nc.scalar.dma_start(
                    w1_sb["A"][:, m : m + 1, :], dram["w1tA"][:, m : m + 1, :]
                )
            for m in range(2, MH):
                nc.gpsimd.dma_start(
                    w1_sb["A"][:, m : m + 1, :], dram["w1tA"][:, m : m + 1, :]
                )
            for ho in range(MH):
                nc.gpsimd.dma_start(
                    w2_sb["A"][:, ho : ho + 1, :], dram["w2tA"][:, ho : ho + 1, :]
                )

            for gi, (s, ti, T, t0) in enumerate(sched):
                # prefetch next tile's x (paced by the xp pool)
                if gi + 1 < len(sched):
                    ns, nti, nT, nt0 = sched[gi + 1]
                    x_tiles[gi + 1] = xp.tile(
                        [P, KO1, T_TILE], bf16, tag="x", name=f"x{gi + 1}"
                    )
                    nc.sync.dma_start(
                        x_tiles[gi + 1][:, :, :nT], xr[ns][:, :, nt0 : nt0 + nT]
                    )
                if gi == 2:
                    # segment-B constants stream on the (slow, software) gpsimd
                    # queue while segment A computes -- it has ~200us of slack.
                    nc.gpsimd.dma_start(b1_sb["B"][:], dram["b1B"][:])
                    for m in range(MH):
                        nc.gpsimd.dma_start(
                            w1_sb["B"][:, m : m + 1, :], dram["w1tB"][:, m : m + 1, :]
                        )
                    for ho in range(MH):
                        nc.gpsimd.dma_start(
                            w2_sb["B"][:, ho : ho + 1, :], dram["w2tB"][:, ho : ho + 1, :]
                        )

                x_sb = x_tiles.pop(gi)
                g_sb = gp.tile([P, MH, T_TILE], bf16, tag="g", name=f"g{gi}")
                for m in range(MH):
                    ph = psum.tile([P, T_TILE], f32, tag="ps", name=f"ph{gi}_{m}")
                    for ko in range(KO1):
                        nc.tensor.matmul(
                            ph[:, :T],
                            w1_sb[s][:, m, ko * P : (ko + 1) * P],
                            x_sb[:, ko, :T],
                            start=(ko == 0),
                            stop=(ko == KO1 - 1),
                        )
                    nc.scalar.activation(
                        g_sb[:, m, :T], ph[:, :T], gelu, bias=b1_sb[s][:, m : m + 1]
                    )
                # layer 2, contraction split in two: the first half only needs
                # g[:8], so it issues right behind the L1 matmuls while the
                # m=15 gelu drains; the second half lands long after.
                pys = [
                    psum.tile([P, T_TILE], f32, tag="ps", name=f"py{gi}_{co}")
                    for co in range(KO1)
                ]
                for co in range(KO1):
                    for ho in range(MH // 2):
                        nc.tensor.matmul(
                            pys[co][:, :T],
                            w2_sb[s][:, ho, co * P : (co + 1) * P],
                            g_sb[:, ho, :T],
                            start=(ho == 0),
                            stop=False,
                        )
                for co in range(KO1):
                    for ho in range(MH // 2, MH):
                        nc.tensor.matmul(
                            pys[co][:, :T],
                            w2_sb[s][:, ho, co * P : (co + 1) * P],
                            g_sb[:, ho, :T],
                            start=False,
                            stop=(ho == MH - 1),
                        )
                    y_sb = yp.tile([P, T_TILE], bf16, tag="y", name=f"y{gi}_{co}")
                    nc.vector.tensor_copy(y_sb[:, :T], pys[co][:, :T])
                    nc.sync.dma_start(yr[s][:, co, t0 : t0 + T], y_sb[:, :T])
    nc.finalize()
    return nc


def _route(flat_f32: np.ndarray, gate_w: np.ndarray):
    """Router, bit-matching the reference's jax ops (same env/backend)."""
    import jax
    import jax.numpy as jnp

    logits = jnp.asarray(flat_f32) @ jnp.asarray(gate_w).T
    probs = jax.nn.softmax(logits, axis=-1)
    top_p, top_i = jax.lax.top_k(probs, TOP_K)
    weights = top_p / (jnp.sum(top_p, axis=-1, keepdims=True) + 1e-8)
    return np.asarray(top_i), np.asarray(weights)


# results of the last device run, for test harness introspection
last_result = None


def _ensure_ntff_hook():
    """bass_utils' trace path imports antenv.axon_hooks, which the agent
    image's antenv lacks. Build the hook from trn_agent_boot's ctypes
    shim and inject a stand-in module."""
    import sys
    import types

    if "antenv.axon_hooks" in sys.modules:
        return
    try:
        from trn_agent_boot.trn_boot import _ntff_profile_via_ctypes

        hook = _ntff_profile_via_ctypes("/opt/axon/libaxon_pjrt.so")
    except Exception:
        hook = None
    m = types.ModuleType("antenv.axon_hooks")
    m.get_axon_ntff_profile_hook = lambda: hook
    m.set_axon_ntff_profile_hook = lambda h: None
    sys.modules["antenv.axon_hooks"] = m


def _prep_w1(w1e_half: np.ndarray) -> np.ndarray:
    # [HH, C] -> [ki, m, ko*128+j] with value w1[m*128+j, ko*128+ki]
    bf16 = ml_dtypes.bfloat16
    a = w1e_half.reshape(MH, P, KO1, P)          # [m, j, ko, ki]
    a = a.transpose(3, 0, 2, 1).reshape(P, MH, C)  # [ki, m, (ko j)]
    return np.ascontiguousarray(a).astype(bf16)


def _prep_w2(w2e_half: np.ndarray) -> np.ndarray:
    # [C, HH] -> [ki, ho, co*128+j] with value w2[co*128+j, ho*128+ki]
    bf16 = ml_dtypes.bfloat16
    a = w2e_half.reshape(KO1, P, MH, P)          # [co, j, ho, ki]
    a = a.transpose(3, 2, 0, 1).reshape(P, MH, C)  # [ki, ho, (co j)]
    return np.ascontiguousarray(a).astype(bf16)


def kernel(x, gate_w, w1, b1, w2, b2):
    from concourse.bass_utils import run_bass_kernel_spmd

    x = np.asarray(x)
    B, N, _ = x.shape
    flat = np.ascontiguousarray(x.reshape(-1, C), dtype=np.float32)
    w1 = np.asarray(w1, dtype=np.float32)
    w2 = np.asarray(w2, dtype=np.float32)
    b1 = np.asarray(b1, dtype=np.float32)
    b2 = np.asarray(b2, dtype=np.float32)

    top_i, weights = _route(flat, np.asarray(gate_w, dtype=np.float32))

    # token ids and combine weights per expert
    idx_e, g_e = [], []
    for e in range(N_EXPERTS):
        rows, cols = np.nonzero(top_i == e)
        idx_e.append(rows)
        g_e.append(weights[rows, cols].astype(np.float32))
    counts = np.array([len(i) for i in idx_e])

    # rank experts by load; segment A = big four, B = small four. Expert
    # ranked[r] runs as two hidden-halves on cores r and r+4; expert
    # ranked[7-r] likewise (segment B on the same core pair).
    ranked = np.argsort(-counts, kind="stable")
    pad = lambda n: max(int(-(-n // P) * P), P)
    capA = pad(int(counts[ranked[0]]))
    capB = pad(int(counts[ranked[4]]))

    key = (capA, capB)
    nc = _nc_cache.get(key)
    if nc is None:
        nc = _build_nc(capA, capB)
        _nc_cache[key] = nc

    bf16 = ml_dtypes.bfloat16

    # per-expert padded x (shared by the expert's two half-cores)
    xt = {}
    for s, cap, exps in (("A", capA, ranked[:4]), ("B", capB, ranked[4:])):
        for e in exps:
            xe = np.zeros((C, cap), dtype=bf16)
            xe[:, : counts[e]] = flat[idx_e[e]].T.astype(bf16)
            xt[int(e)] = xe

    in_maps = []
    for core in range(8):
        r, half = core % 4, core // 4
        m = {}
        for s, r_e in (("A", ranked[r]), ("B", ranked[7 - r])):
            e = int(r_e)
            lo, hi = half * HH, (half + 1) * HH
            m[f"xt{s}"] = xt[e]
            m[f"w1t{s}"] = _prep_w1(w1[e, lo:hi, :])
            m[f"w2t{s}"] = _prep_w2(w2[e, :, lo:hi])
            m[f"b1{s}"] = np.ascontiguousarray(b1[e, lo:hi].reshape(MH, P).T)
        in_maps.append(m)

    import os

    trace = bool(int(os.environ.get("MOE_TRACE", "0")))
    if trace:
        _ensure_ntff_hook()

    global last_result
    res = run_bass_kernel_spmd(
        nc,
        in_maps,
        core_ids=list(range(8)),
        trace=trace,
    )
    last_result = res

    out = np.zeros((flat.shape[0], C), dtype=np.float32)
    for r in range(4):
        for s, r_e in (("A", ranked[r]), ("B", ranked[7 - r])):
            e = int(r_e)
            cnt = counts[e]
            y = res.results[r][f"yt{s}"][:, :cnt].astype(np.float32)
            y += res.results[r + 4][f"yt{s}"][:, :cnt].astype(np.float32)
            out[idx_e[e]] += g_e[e][:, None] * (y.T + b2[e])
    return out.reshape(B, N, C)
